# Initial kernel scaffold
#
"""LightGCN-style 3-layer graph propagation on 8 Trainium2 NeuronCores.

Strategy (dest-sharded nodes, source-block-grouped edges):
  - Nodes are sharded across 8 cores by destination row (25k rows/core).
  - Each core's rows are packed into 128-row "windows" (PSUM tiles), with a
    greedy balancer that caps the edge count of every (window, source-block)
    cell at CHUNKS*128 so the SPMD program is identical on all cores.
  - Edge messages x[col] are fetched with SWDGE dma_gather (256B rows) from a
    replicated fp16 table whose rows are duplicated ([x; x]) to satisfy the
    256B-multiple element-size constraint.
  - The per-window segment-sum is a one-hot matmul: for each 128-edge chunk,
    a val-weighted one-hot lhsT is built with ONE dual-op DVE tensor_scalar
    (is_equal then mult) against a constant iota row, then TensorE contracts
    the chunk's 128 messages into the window's PSUM tile (fp32 accumulate).
  - Between layers the 25k-row fp16 output shards are AllGathered (ncfw
    collective) back into the full 200k-row table.
  - acc = ego1+ego2+ego3 is kept on-chip (fp16); the host divides by 3,
    un-permutes the device row order and casts back to fp32.
"""

import sys

if "/opt/trn_rl_repo" not in sys.path:
    sys.path.insert(0, "/opt/trn_rl_repo")

import numpy as np

# ----------------------------------------------------------------------------
# configs
# ----------------------------------------------------------------------------

def make_cfg(shard_rows, nwin, wps, nnz, n_layers=3, chunks=3, cores=8, emb=64):
    assert nwin * 128 >= shard_rows > (nwin - 1) * 128
    assert nwin % wps == 0
    cfg = dict(
        CORES=cores,
        EMB=emb,
        SHARD=shard_rows,          # real rows per core
        NWIN=nwin,                 # 128-row windows per core
        WPS=wps,                   # windows per super-block
        NSUP=nwin // wps,          # super-blocks
        CB=chunks,                 # 128-edge chunks per (window, group)
        NNZ=nnz,
        LAYERS=n_layers,
        DR=nwin * 128,             # device rows per core (incl. dummy slots)
    )
    cfg["N"] = cores * shard_rows
    cfg["CALL_IDX"] = wps * chunks * 128      # gather indices per call
    assert cfg["DR"] <= 32767, "int16 gather index overflow"
    return cfg


FULL_CFG = make_cfg(shard_rows=25000, nwin=196, wps=7, nnz=4_000_000)
USER_NUM = 100_000


# ----------------------------------------------------------------------------
# host-side preprocessing
# ----------------------------------------------------------------------------

def _assign_windows(deg, cfg, rng):
    """Pack rows (local ids) of one core into NWIN windows of <=128 rows so
    that every (window, group) edge count stays <= CB*128.

    deg: [SHARD, CORES] per-group degree of each row.
    Returns win_of[SHARD], slot_of[SHARD] (partition within window).
    """
    nwin, cap = cfg["NWIN"], cfg["CB"] * 128
    shard, cores = deg.shape
    loads = np.zeros((nwin, cores), dtype=np.int64)
    counts = np.zeros(nwin, dtype=np.int64)
    win_of = np.full(shard, -1, dtype=np.int64)
    slot_of = np.full(shard, -1, dtype=np.int64)
    order = np.argsort(-deg.sum(1), kind="stable")
    for r in order:
        d = deg[r]
        ok = (counts < 128) & (loads + d <= cap).all(axis=1)
        if not ok.any():
            raise RuntimeError("window packing infeasible; raise CB")
        # among feasible windows pick the one with the smallest resulting
        # max cell load (break ties by emptiest window)
        cand = np.where(ok)[0]
        score = (loads[cand] + d).max(axis=1) * 1000 + counts[cand]
        w = cand[np.argmin(score)]
        win_of[r] = w
        slot_of[r] = counts[w]
        counts[w] += 1
        loads[w] += d
    return win_of, slot_of


def prepare_host_data(cfg, x0, adj_row, adj_col, adj_val):
    """Build all per-core device inputs. Returns (in_maps_extra, node_of)."""
    cores, shard, nwin, wps, cb = (
        cfg["CORES"], cfg["SHARD"], cfg["NWIN"], cfg["WPS"], cfg["CB"])
    nsup, dr, emb = cfg["NSUP"], cfg["DR"], cfg["EMB"]
    slots_wb = cb * 128

    r = np.asarray(adj_row, dtype=np.int64)
    c = np.asarray(adj_col, dtype=np.int64)
    v = np.asarray(adj_val, dtype=np.float32)
    dest_core = r // shard
    src_core = c // shard

    rng = np.random.default_rng(0)

    # ---- window assignment per core (also yields global device positions)
    win_of = np.zeros(cfg["N"], dtype=np.int64)   # window of each global node
    part_of = np.zeros(cfg["N"], dtype=np.int64)  # partition within window
    for cid in range(cores):
        lo = cid * shard
        sel = dest_core == cid
        rl = r[sel] - lo
        deg = np.zeros((shard, cores), dtype=np.int64)
        np.add.at(deg, (rl, src_core[sel]), 1)
        w, s = _assign_windows(deg, cfg, rng)
        win_of[lo:lo + shard] = w
        part_of[lo:lo + shard] = s

    # device-local index of every node inside its own core block (p-major)
    devloc = part_of * nwin + win_of            # in [0, DR)

    # node_of[cid][w, p] = global node id (or -1)
    node_of = np.full((cores, nwin, 128), -1, dtype=np.int64)
    for cid in range(cores):
        lo = cid * shard
        ids = np.arange(lo, lo + shard)
        node_of[cid, win_of[ids], part_of[ids]] = ids

    # ---- fp16 duplicated node table (device order, replicated to all cores)
    x0_dev = np.zeros((cores * dr, 2 * emb), dtype=np.float16)
    for cid in range(cores):
        ids = np.where(node_of[cid].reshape(-1) >= 0)[0]
        gids = node_of[cid].reshape(-1)[ids]
        xh = x0[gids].astype(np.float16)
        x0_dev[cid * dr + ids, :emb] = xh
        x0_dev[cid * dr + ids, emb:] = xh

    # ---- per-core edge slotting
    in_extras = []
    ncalls = nsup * cores
    for cid in range(cores):
        sel = dest_core == cid
        er, ec, ev = r[sel], c[sel], v[sel]
        ew = win_of[er]                     # dest window
        ep = part_of[er]                    # dest partition (one-hot target)
        eb = src_core[sel]                  # source block/group
        esrc = devloc[ec]                   # gather index within block

        # order edges by (window, group); slot within each (w,b) cell
        key = ew * cores + eb
        order = np.argsort(key, kind="stable")
        key_s = key[order]
        cell_cnt = np.bincount(key_s, minlength=nwin * cores)
        if cell_cnt.max() > slots_wb:
            raise RuntimeError("cell overflow after packing")
        cell_base = np.zeros(nwin * cores, dtype=np.int64)
        # slot index within cell
        slot_in_cell = np.arange(len(key_s)) - np.repeat(
            np.concatenate([[0], np.cumsum(cell_cnt)[:-1]]), cell_cnt)

        # dense per-slot arrays, padded with idx=0/dest=0/val=0
        idx_arr = np.zeros((nwin, cores, slots_wb), dtype=np.int16)
        dest_arr = np.zeros((nwin, cores, slots_wb), dtype=np.float32)
        val_arr = np.zeros((nwin, cores, slots_wb), dtype=np.float32)
        wv, bv = key_s // cores, key_s % cores
        idx_arr[wv, bv, slot_in_cell] = esrc[order].astype(np.int16)
        dest_arr[wv, bv, slot_in_cell] = ep[order].astype(np.float32)
        val_arr[wv, bv, slot_in_cell] = ev[order]

        # gather call layout: call (s, b) covers windows [s*wps, (s+1)*wps).
        # list position i -> partition i%128, column i//128; columns ordered
        # (w_local, chunk). wrapped-16 then replicated to 128 partitions.
        ci = cfg["CALL_IDX"]
        idx_dev = np.zeros((ncalls, 128, ci // 16), dtype=np.int16)
        for s in range(nsup):
            for b in range(cores):
                # [wps, cb, 128] -> flat list
                lst = idx_arr[s * wps:(s + 1) * wps, b, :].reshape(
                    wps, cb, 128).reshape(-1)
                wrapped = lst.reshape(ci // 16, 16).T  # [16, ci//16]
                idx_dev[s * cores + b] = np.tile(wrapped, (8, 1))

        # dest/val scalar tiles: [128, NWIN, cores*CB] (partition = slot%128)
        dest_dev = dest_arr.reshape(nwin, cores, cb, 128).transpose(3, 0, 1, 2) \
            .reshape(128, nwin, cores * cb).copy()
        val_dev = val_arr.reshape(nwin, cores, cb, 128).transpose(3, 0, 1, 2) \
            .reshape(128, nwin, cores * cb).copy()

        iota = np.tile(np.arange(128, dtype=np.float16), (128, 1))

        in_extras.append({
            "x0_dev": x0_dev,
            "idx_dev": idx_dev,
            "dest_dev": dest_dev,
            "val_dev": val_dev,
            "iota_dev": iota,
        })
    return in_extras, node_of


# ----------------------------------------------------------------------------
# device program
# ----------------------------------------------------------------------------

def build_bass(cfg, debug=False):
    import concourse.bacc as bacc
    import concourse.bass as bass
    import concourse.mybir as mybir
    import concourse.tile as tile
    from contextlib import ExitStack

    cores, nwin, wps, cb = cfg["CORES"], cfg["NWIN"], cfg["WPS"], cfg["CB"]
    nsup, dr, emb, layers = cfg["NSUP"], cfg["DR"], cfg["EMB"], cfg["LAYERS"]
    ci = cfg["CALL_IDX"]
    f16, f32, i16 = mybir.dt.float16, mybir.dt.float32, mybir.dt.int16
    K = cores * cb                     # chunks (and matmuls) per window

    nc = bacc.Bacc("TRN2", target_bir_lowering=False, debug=debug,
                   num_devices=cores)

    x0_dev = nc.dram_tensor("x0_dev", [cores * dr, 2 * emb], f16,
                            kind="ExternalInput")
    idx_dev = nc.dram_tensor("idx_dev", [nsup * cores, 128, ci // 16], i16,
                             kind="ExternalInput")
    dest_dev = nc.dram_tensor("dest_dev", [128, nwin, K], f32,
                              kind="ExternalInput")
    val_dev = nc.dram_tensor("val_dev", [128, nwin, K], f32,
                             kind="ExternalInput")
    iota_dev = nc.dram_tensor("iota_dev", [128, 128], f16,
                              kind="ExternalInput")
    out_acc = nc.dram_tensor("out_acc", [128, nwin, emb], f16,
                             kind="ExternalOutput")

    ag_in = [nc.dram_tensor(f"ag_in{l}", [dr, 2 * emb], f16)
             for l in range(layers - 1)]
    ag_out = [nc.dram_tensor(f"ag_out{l}", [cores * dr, 2 * emb], f16,
                             addr_space="Shared")
              for l in range(layers - 1)]

    with tile.TileContext(nc) as tc, ExitStack() as ex:
        const_p = ex.enter_context(tc.tile_pool(name="const", bufs=1))
        idx_p = ex.enter_context(tc.tile_pool(name="idx", bufs=4))
        sc_p = ex.enter_context(tc.tile_pool(name="sc", bufs=2))
        g_p = ex.enter_context(tc.tile_pool(name="g", bufs=2))
        pt_p = ex.enter_context(tc.tile_pool(name="pt", bufs=3))
        ps_p = ex.enter_context(tc.tile_pool(name="ps", bufs=2, space="PSUM"))
        big_p = ex.enter_context(tc.tile_pool(name="big", bufs=1))

        iota_t = const_p.tile([128, 128], f16)
        nc.sync.dma_start(iota_t[:, :], iota_dev[:, :])

        acc_t = big_p.tile([128, nwin, emb], f16, tag="acc")
        nc.vector.memset(acc_t[:, :, :], 0.0)
        y_t = big_p.tile([128, nwin, emb], f16, tag="y")

        eq, mul = mybir.AluOpType.is_equal, mybir.AluOpType.mult

        for l in range(layers):
            x_src = x0_dev if l == 0 else ag_out[l - 1]
            for s in range(nsup):
                gts = []
                for b in range(cores):
                    it = idx_p.tile([128, ci // 16], i16, tag="idx")
                    nc.sync.dma_start(it[:, :], idx_dev[s * cores + b, :, :])
                    gt = g_p.tile([128, wps * cb, 2 * emb], f16, tag=f"g{b}")
                    nc.gpsimd.dma_gather(
                        gt[:, :, :], x_src[b * dr:(b + 1) * dr, :],
                        it[:, :], ci, ci, 2 * emb)
                    gts.append(gt)
                dt = sc_p.tile([128, wps, K], f32, tag="dest")
                nc.sync.dma_start(dt[:, :, :],
                                  dest_dev[:, s * wps:(s + 1) * wps, :])
                vt = sc_p.tile([128, wps, K], f32, tag="val")
                nc.sync.dma_start(vt[:, :, :],
                                  val_dev[:, s * wps:(s + 1) * wps, :])

                ps = ps_p.tile([128, wps, emb], f32, tag="ps")
                for wl in range(wps):
                    pt = pt_p.tile([128, K, 128], f16, tag="pt")
                    for k in range(K):
                        nc.vector.tensor_scalar(
                            out=pt[:, k, :], in0=iota_t[:, :],
                            scalar1=dt[:, wl, k], scalar2=vt[:, wl, k],
                            op0=eq, op1=mul)
                    for b in range(cores):
                        for ch in range(cb):
                            k = b * cb + ch
                            nc.tensor.matmul(
                                ps[:, wl, :],
                                lhsT=pt[:, k, :],
                                rhs=gts[b][:, wl * cb + ch, 0:emb],
                                start=(k == 0), stop=(k == K - 1))
                # evacuate: acc += psum (fp16), y = psum (fp16)
                sl = slice(s * wps, (s + 1) * wps)
                nc.vector.tensor_tensor(
                    out=acc_t[:, sl, :], in0=ps[:, :, :],
                    in1=acc_t[:, sl, :], op=mybir.AluOpType.add)
                if l < layers - 1:
                    nc.scalar.activation(
                        y_t[:, sl, :], ps[:, :, :],
                        mybir.ActivationFunctionType.Copy)
            if l < layers - 1:
                # ship y (duplicated halves) and all-gather into next table
                agv = ag_in[l].ap().rearrange("(p w) e -> p w e", p=128)
                nc.sync.dma_start(agv[:, :, 0:emb], y_t[:, :, :])
                nc.sync.dma_start(agv[:, :, emb:2 * emb], y_t[:, :, :])
                nc.gpsimd.collective_compute(
                    "AllGather",
                    mybir.AluOpType.bypass,
                    ins=[ag_in[l].ap().opt()],
                    outs=[ag_out[l].ap().opt()],
                    replica_groups=[list(range(cores))],
                )
        nc.sync.dma_start(out_acc[:, :, :], acc_t[:, :, :])

    nc.compile()
    return nc


# ----------------------------------------------------------------------------
# top-level entry
# ----------------------------------------------------------------------------

def run(cfg, user_emb, item_emb, adj_row, adj_col, adj_val,
        sim=False, trace=False, debug=False):
    from concourse.bass_utils import run_bass_kernel_spmd

    x0 = np.concatenate([np.asarray(user_emb, np.float32),
                         np.asarray(item_emb, np.float32)], axis=0)
    in_extras, node_of = prepare_host_data(cfg, x0, adj_row, adj_col, adj_val)
    nc = build_bass(cfg, debug=debug)

    cores, nwin, emb, shard = cfg["CORES"], cfg["NWIN"], cfg["EMB"], cfg["SHARD"]
    core_ids = list(range(cores))

    if sim:
        from concourse.bass_interp import MultiCoreSim
        msim = MultiCoreSim(nc, num_cores=cores)
        for cid in range(cores):
            for k, a in in_extras[cid].items():
                msim.cores[cid].tensor(k)[:] = a
        msim.simulate(check_with_hw=False)
        outs = [np.array(msim.cores[cid].mem_tensor("out_acc"))
                for cid in range(cores)]
        res = None
    else:
        in_maps = [dict(in_extras[cid]) for cid in range(cores)]
        res = run_bass_kernel_spmd(nc, in_maps, core_ids, trace=trace,
                                   trace_cores=core_ids if trace else None)
        outs = [res.results[i]["out_acc"] for i in range(cores)]

    final = np.zeros((cfg["N"], emb), dtype=np.float32)
    for cid in range(cores):
        o = np.asarray(outs[cid], dtype=np.float32)  # [128, nwin, emb]
        valid = node_of[cid] >= 0                    # [nwin, 128]
        w_idx, p_idx = np.nonzero(valid)
        final[node_of[cid][w_idx, p_idx]] = o[p_idx, w_idx, :]
    final /= cfg["LAYERS"]
    return final, res


def kernel(user_emb, item_emb, adj_row, adj_col, adj_val):
    final, _ = run(FULL_CFG, user_emb, item_emb, adj_row, adj_col, adj_val)
    return final[:USER_NUM], final[USER_NUM:]


# revision 4
# speedup vs baseline: 48.1952x; 48.1952x over previous
"""LightGCN-style 3-layer graph propagation on 8 Trainium2 NeuronCores.

Strategy (dest-sharded nodes, source-block-grouped edges):
  - Nodes are sharded across 8 cores by destination row (25k rows/core).
  - Each core's rows are packed into 128-row "windows" (PSUM tiles), with a
    greedy balancer that caps the edge count of every (window, source-block)
    cell at CHUNKS*128 so the SPMD program is identical on all cores.
  - Edge messages x[col] are fetched with SWDGE dma_gather (256B rows) from a
    replicated fp16 table whose rows are duplicated ([x; x]) to satisfy the
    256B-multiple element-size constraint.
  - The per-window segment-sum is a one-hot matmul: for each 128-edge chunk,
    a val-weighted one-hot lhsT is built with ONE dual-op DVE tensor_scalar
    (is_equal then mult) against a constant iota row, then TensorE contracts
    the chunk's 128 messages into the window's PSUM tile (fp32 accumulate).
  - Between layers the 25k-row fp16 output shards are AllGathered (ncfw
    collective) back into the full 200k-row table.
  - acc = ego1+ego2+ego3 is kept on-chip (fp16); the host divides by 3,
    un-permutes the device row order and casts back to fp32.
"""

import sys

if "/opt/trn_rl_repo" not in sys.path:
    sys.path.insert(0, "/opt/trn_rl_repo")

import numpy as np

# ----------------------------------------------------------------------------
# configs
# ----------------------------------------------------------------------------

def make_cfg(shard_rows, nwin, wps, nnz, n_layers=3, chunks=3, cores=8, emb=64):
    assert nwin * 128 >= shard_rows > (nwin - 1) * 128
    assert nwin % wps == 0
    cfg = dict(
        CORES=cores,
        EMB=emb,
        SHARD=shard_rows,          # real rows per core
        NWIN=nwin,                 # 128-row windows per core
        WPS=wps,                   # windows per super-block
        NSUP=nwin // wps,          # super-blocks
        CB=chunks,                 # 128-edge chunks per (window, group)
        NNZ=nnz,
        LAYERS=n_layers,
        DR=nwin * 128,             # device rows per core (incl. dummy slots)
    )
    cfg["N"] = cores * shard_rows
    cfg["CALL_IDX"] = wps * chunks * 128      # gather indices per call
    assert cfg["DR"] <= 32767, "int16 gather index overflow"
    return cfg


FULL_CFG = make_cfg(shard_rows=25000, nwin=196, wps=7, nnz=4_000_000)
USER_NUM = 100_000


# ----------------------------------------------------------------------------
# host-side preprocessing
# ----------------------------------------------------------------------------

def _assign_windows(deg, cfg, rng):
    """Pack rows (local ids) of one core into NWIN windows of <=128 rows so
    that every (window, group) edge count stays <= CB*128.

    deg: [SHARD, CORES] per-group degree of each row.
    Returns win_of[SHARD], slot_of[SHARD] (partition within window).
    """
    nwin, cap = cfg["NWIN"], cfg["CB"] * 128
    shard, cores = deg.shape
    loads = np.zeros((nwin, cores), dtype=np.int64)
    counts = np.zeros(nwin, dtype=np.int64)
    win_of = np.full(shard, -1, dtype=np.int64)
    slot_of = np.full(shard, -1, dtype=np.int64)
    order = np.argsort(-deg.sum(1), kind="stable")
    for r in order:
        d = deg[r]
        ok = (counts < 128) & (loads + d <= cap).all(axis=1)
        if not ok.any():
            raise RuntimeError("window packing infeasible; raise CB")
        # among feasible windows pick the one with the smallest resulting
        # max cell load (break ties by emptiest window)
        cand = np.where(ok)[0]
        score = (loads[cand] + d).max(axis=1) * 1000 + counts[cand]
        w = cand[np.argmin(score)]
        win_of[r] = w
        slot_of[r] = counts[w]
        counts[w] += 1
        loads[w] += d
    return win_of, slot_of


def prepare_host_data(cfg, x0, adj_row, adj_col, adj_val):
    """Build all per-core device inputs. Returns (in_maps_extra, node_of)."""
    cores, shard, nwin, wps, cb = (
        cfg["CORES"], cfg["SHARD"], cfg["NWIN"], cfg["WPS"], cfg["CB"])
    nsup, dr, emb = cfg["NSUP"], cfg["DR"], cfg["EMB"]
    slots_wb = cb * 128

    r = np.asarray(adj_row, dtype=np.int64)
    c = np.asarray(adj_col, dtype=np.int64)
    v = np.asarray(adj_val, dtype=np.float32)
    dest_core = r // shard
    src_core = c // shard

    rng = np.random.default_rng(0)

    # ---- window assignment per core (also yields global device positions)
    win_of = np.zeros(cfg["N"], dtype=np.int64)   # window of each global node
    part_of = np.zeros(cfg["N"], dtype=np.int64)  # partition within window
    for cid in range(cores):
        lo = cid * shard
        sel = dest_core == cid
        rl = r[sel] - lo
        deg = np.zeros((shard, cores), dtype=np.int64)
        np.add.at(deg, (rl, src_core[sel]), 1)
        w, s = _assign_windows(deg, cfg, rng)
        win_of[lo:lo + shard] = w
        part_of[lo:lo + shard] = s

    # device-local index of every node inside its own core block (p-major)
    devloc = part_of * nwin + win_of            # in [0, DR)

    # node_of[cid][w, p] = global node id (or -1)
    node_of = np.full((cores, nwin, 128), -1, dtype=np.int64)
    for cid in range(cores):
        lo = cid * shard
        ids = np.arange(lo, lo + shard)
        node_of[cid, win_of[ids], part_of[ids]] = ids

    # ---- fp16 duplicated node table (device order, replicated to all cores)
    # device row order is p-major: row = p * nwin + w (matches the SBUF
    # [partition, window, emb] layout the per-layer output DMA produces)
    x0_dev = np.zeros((cores * dr, 2 * emb), dtype=np.float16)
    for cid in range(cores):
        w_idx, p_idx = np.nonzero(node_of[cid] >= 0)
        gids = node_of[cid][w_idx, p_idx]
        xh = x0[gids].astype(np.float16)
        rows = cid * dr + p_idx * nwin + w_idx
        x0_dev[rows, :emb] = xh
        x0_dev[rows, emb:] = xh

    # ---- per-core edge slotting
    in_extras = []
    ncalls = nsup * cores
    for cid in range(cores):
        sel = dest_core == cid
        er, ec, ev = r[sel], c[sel], v[sel]
        ew = win_of[er]                     # dest window
        ep = part_of[er]                    # dest partition (one-hot target)
        eb = src_core[sel]                  # source block/group
        esrc = devloc[ec]                   # gather index within block

        # order edges by (window, group); slot within each (w,b) cell
        key = ew * cores + eb
        order = np.argsort(key, kind="stable")
        key_s = key[order]
        cell_cnt = np.bincount(key_s, minlength=nwin * cores)
        if cell_cnt.max() > slots_wb:
            raise RuntimeError("cell overflow after packing")
        cell_base = np.zeros(nwin * cores, dtype=np.int64)
        # slot index within cell
        slot_in_cell = np.arange(len(key_s)) - np.repeat(
            np.concatenate([[0], np.cumsum(cell_cnt)[:-1]]), cell_cnt)

        # dense per-slot arrays, padded with idx=0/dest=0/val=0
        idx_arr = np.zeros((nwin, cores, slots_wb), dtype=np.int16)
        dest_arr = np.zeros((nwin, cores, slots_wb), dtype=np.float32)
        val_arr = np.zeros((nwin, cores, slots_wb), dtype=np.float32)
        wv, bv = key_s // cores, key_s % cores
        idx_arr[wv, bv, slot_in_cell] = esrc[order].astype(np.int16)
        dest_arr[wv, bv, slot_in_cell] = ep[order].astype(np.float32)
        val_arr[wv, bv, slot_in_cell] = ev[order]

        # gather call layout: call (s, b) covers windows [s*wps, (s+1)*wps).
        # list position i -> partition i%128, column i//128; columns ordered
        # (w_local, chunk). wrapped-16 then replicated to 128 partitions.
        ci = cfg["CALL_IDX"]
        idx_dev = np.zeros((ncalls, 128, ci // 16), dtype=np.int16)
        for s in range(nsup):
            for b in range(cores):
                # [wps, cb, 128] -> flat list
                lst = idx_arr[s * wps:(s + 1) * wps, b, :].reshape(
                    wps, cb, 128).reshape(-1)
                wrapped = lst.reshape(ci // 16, 16).T  # [16, ci//16]
                idx_dev[s * cores + b] = np.tile(wrapped, (8, 1))

        # dest/val scalar tiles: [128, NWIN, cores*CB] (partition = slot%128)
        dest_dev = dest_arr.reshape(nwin, cores, cb, 128).transpose(3, 0, 1, 2) \
            .reshape(128, nwin, cores * cb).copy()
        val_dev = val_arr.reshape(nwin, cores, cb, 128).transpose(3, 0, 1, 2) \
            .reshape(128, nwin, cores * cb).copy()

        iota = np.tile(np.arange(128, dtype=np.float16), (128, 1))

        in_extras.append({
            "x0_dev": x0_dev,
            "idx_dev": idx_dev,
            "dest_dev": dest_dev,
            "val_dev": val_dev,
            "iota_dev": iota,
        })
    return in_extras, node_of


# ----------------------------------------------------------------------------
# device program
# ----------------------------------------------------------------------------

def build_bass(cfg, debug=False):
    import concourse.bacc as bacc
    import concourse.bass as bass
    import concourse.mybir as mybir
    import concourse.tile as tile
    from contextlib import ExitStack

    cores, nwin, wps, cb = cfg["CORES"], cfg["NWIN"], cfg["WPS"], cfg["CB"]
    nsup, dr, emb, layers = cfg["NSUP"], cfg["DR"], cfg["EMB"], cfg["LAYERS"]
    ci = cfg["CALL_IDX"]
    f16, f32, i16 = mybir.dt.float16, mybir.dt.float32, mybir.dt.int16
    K = cores * cb                     # chunks (and matmuls) per window

    nc = bacc.Bacc("TRN2", target_bir_lowering=False, debug=debug,
                   num_devices=cores)

    x0_dev = nc.dram_tensor("x0_dev", [cores * dr, 2 * emb], f16,
                            kind="ExternalInput")
    idx_dev = nc.dram_tensor("idx_dev", [nsup * cores, 128, ci // 16], i16,
                             kind="ExternalInput")
    dest_dev = nc.dram_tensor("dest_dev", [128, nwin, K], f32,
                              kind="ExternalInput")
    val_dev = nc.dram_tensor("val_dev", [128, nwin, K], f32,
                             kind="ExternalInput")
    iota_dev = nc.dram_tensor("iota_dev", [128, 128], f16,
                              kind="ExternalInput")
    out_acc = nc.dram_tensor("out_acc", [128, nwin, emb], f16,
                             kind="ExternalOutput")

    ag_in = [nc.dram_tensor(f"ag_in{l}", [dr, 2 * emb], f16)
             for l in range(layers - 1)]
    ag_out = [nc.dram_tensor(f"ag_out{l}", [cores * dr, 2 * emb], f16,
                             addr_space="Shared")
              for l in range(layers - 1)]

    with tile.TileContext(nc) as tc, ExitStack() as ex:
        const_p = ex.enter_context(tc.tile_pool(name="const", bufs=1))
        idx_p = ex.enter_context(tc.tile_pool(name="idx", bufs=4))
        sc_p = ex.enter_context(tc.tile_pool(name="sc", bufs=2))
        g_p = ex.enter_context(tc.tile_pool(name="g", bufs=2))
        pt_p = ex.enter_context(tc.tile_pool(name="pt", bufs=3))
        ps_p = ex.enter_context(tc.tile_pool(name="ps", bufs=2, space="PSUM"))
        big_p = ex.enter_context(tc.tile_pool(name="big", bufs=1))

        iota_t = const_p.tile([128, 128], f16)
        nc.sync.dma_start(iota_t[:, :], iota_dev[:, :])

        acc_t = big_p.tile([128, nwin, emb], f16, tag="acc")
        nc.vector.memset(acc_t[:, :, :], 0.0)
        y_t = big_p.tile([128, nwin, emb], f16, tag="y")

        eq, mul = mybir.AluOpType.is_equal, mybir.AluOpType.mult

        for l in range(layers):
            x_src = x0_dev if l == 0 else ag_out[l - 1]
            for s in range(nsup):
                gts = []
                for b in range(cores):
                    it = idx_p.tile([128, ci // 16], i16, tag="idx")
                    nc.sync.dma_start(it[:, :], idx_dev[s * cores + b, :, :])
                    gt = g_p.tile([128, wps * cb, 2 * emb], f16, tag=f"g{b}")
                    nc.gpsimd.dma_gather(
                        gt[:, :, :], x_src[b * dr:(b + 1) * dr, :],
                        it[:, :], ci, ci, 2 * emb)
                    gts.append(gt)
                dt = sc_p.tile([128, wps, K], f32, tag="dest")
                nc.sync.dma_start(dt[:, :, :],
                                  dest_dev[:, s * wps:(s + 1) * wps, :])
                vt = sc_p.tile([128, wps, K], f32, tag="val")
                nc.sync.dma_start(vt[:, :, :],
                                  val_dev[:, s * wps:(s + 1) * wps, :])

                ps = ps_p.tile([128, wps, emb], f32, tag="ps")
                for wl in range(wps):
                    pt = pt_p.tile([128, K, 128], f16, tag="pt")
                    for k in range(K):
                        nc.vector.tensor_scalar(
                            out=pt[:, k, :], in0=iota_t[:, :],
                            scalar1=dt[:, wl, k:k + 1],
                            scalar2=vt[:, wl, k:k + 1],
                            op0=eq, op1=mul)
                    for b in range(cores):
                        for ch in range(cb):
                            k = b * cb + ch
                            nc.tensor.matmul(
                                ps[:, wl, :],
                                lhsT=pt[:, k, :],
                                rhs=gts[b][:, wl * cb + ch, 0:emb],
                                start=(k == 0), stop=(k == K - 1))
                # evacuate: acc += psum (fp16), y = psum (fp16)
                sl = slice(s * wps, (s + 1) * wps)
                nc.vector.tensor_tensor(
                    out=acc_t[:, sl, :], in0=ps[:, :, :],
                    in1=acc_t[:, sl, :], op=mybir.AluOpType.add)
                if l < layers - 1:
                    nc.scalar.activation(
                        y_t[:, sl, :], ps[:, :, :],
                        mybir.ActivationFunctionType.Copy)
            if l < layers - 1:
                # ship y (duplicated halves) and all-gather into next table
                agv = ag_in[l].ap().rearrange("(p w) e -> p w e", p=128)
                nc.sync.dma_start(agv[:, :, 0:emb], y_t[:, :, :])
                nc.sync.dma_start(agv[:, :, emb:2 * emb], y_t[:, :, :])
                nc.gpsimd.collective_compute(
                    "AllGather",
                    mybir.AluOpType.bypass,
                    ins=[ag_in[l].ap().opt()],
                    outs=[ag_out[l].ap().opt()],
                    replica_groups=[list(range(cores))],
                )
        nc.sync.dma_start(out_acc[:, :, :], acc_t[:, :, :])

    nc.compile()
    return nc


# ----------------------------------------------------------------------------
# top-level entry
# ----------------------------------------------------------------------------

def run(cfg, user_emb, item_emb, adj_row, adj_col, adj_val,
        sim=False, trace=False, debug=False):
    from concourse.bass_utils import run_bass_kernel_spmd

    x0 = np.concatenate([np.asarray(user_emb, np.float32),
                         np.asarray(item_emb, np.float32)], axis=0)
    in_extras, node_of = prepare_host_data(cfg, x0, adj_row, adj_col, adj_val)
    nc = build_bass(cfg, debug=debug)

    cores, nwin, emb, shard = cfg["CORES"], cfg["NWIN"], cfg["EMB"], cfg["SHARD"]
    core_ids = list(range(cores))

    if sim:
        from concourse.bass_interp import MultiCoreSim
        msim = MultiCoreSim(nc, num_cores=cores)
        for cid in range(cores):
            for k, a in in_extras[cid].items():
                msim.cores[cid].tensor(k)[:] = a
        msim.simulate(check_with_hw=False)
        outs = [np.array(msim.cores[cid].mem_tensor("out_acc"))
                for cid in range(cores)]
        res = None
    else:
        in_maps = [dict(in_extras[cid]) for cid in range(cores)]
        res = run_bass_kernel_spmd(nc, in_maps, core_ids, trace=trace,
                                   trace_cores=core_ids if trace else None)
        outs = [res.results[i]["out_acc"] for i in range(cores)]

    final = np.zeros((cfg["N"], emb), dtype=np.float32)
    for cid in range(cores):
        o = np.asarray(outs[cid], dtype=np.float32).reshape(128, nwin, emb)
        valid = node_of[cid] >= 0                    # [nwin, 128]
        w_idx, p_idx = np.nonzero(valid)
        final[node_of[cid][w_idx, p_idx]] = o[p_idx, w_idx, :]
    final /= cfg["LAYERS"]
    return final, res


def kernel(user_emb, item_emb, adj_row, adj_col, adj_val):
    final, _ = run(FULL_CFG, user_emb, item_emb, adj_row, adj_col, adj_val)
    return final[:USER_NUM], final[USER_NUM:]
